# revision 32
# baseline (speedup 1.0000x reference)
"""ComplexCrossAttention Trainium2 kernel: 8 cores = DP(batch=2) x TP(head-groups=4).

Each core (b = core//4, g = core%4) handles batch b and heads 4g..4g+3.
All matmuls run in bf16 with fp32 PSUM accumulation (fp8 fails precision
here: scores have std ~5.2 so softmax is near-argmax, which amplifies any
score noise into top-key rank flips and passes V noise through unaveraged).

Structure (v3), tuned to keep the PE streaming at its bf16 column-rate
floor:
- K/V projections run first (ctx is small, DMA'd on the gpsimd queue) so
  the 8MB xT stream on the sync queue never starves the PE.
- complex arithmetic is folded into matmul chains by packing weights
  host-side: per head the on-chip Q/K layout is [Qr_h(64); Qi_h(64)] rows
  so scores_h^T = Kx_h^T @ Qx_h in one K=128 matmul per k-tile.
- scores live transposed ([k, q]); the softmax mask is a per-partition
  activation bias.
- the softmax denominator leaves the PE: the DVE sums the 8 exp tiles
  pairwise (bf16) and a single ones-matmul per (head, q-tile) reduces the
  partition dim; reciprocal via the fast custom-DVE op.
- out-projection chains are interleaved between score matmuls of the next
  q tile (PE is in-order; the independent chains fill the wait for the
  Scalar engine's exp), with Q-proj chains of q2/q3 as the filler during
  q0. y is written bf16; the host adds the per-core partials in f32 (the
  hint's all-reduce).
"""

import numpy as np
import ml_dtypes

import concourse.bacc as bacc
import concourse.mybir as mybir
import concourse.tile as tile
from concourse.bass_utils import run_bass_kernel_spmd

BF16 = ml_dtypes.bfloat16
F32 = mybir.dt.float32
BF = mybir.dt.bfloat16

B, S, Lc = 2, 2048, 1024
F, Dc, H = 1024, 768, 16
HD = 64
NCORES = 8
TPG = 4            # head-groups (TP degree per batch)
FS = F // TPG      # 256 features per core
HL = 4             # heads per core
NQ, QTS = 4, 512   # q tiles
NKT = 8            # k tiles of 128 (Lc)
NFIN = 8           # f_in chunks of 128 (Q proj contraction)
NDC = 6            # Dc chunks of 128 (K/V proj contraction)
WW = 2 * HD * HL   # 512 merged (r,i) weight columns per core
SCALE = 1.0 / 8.0  # 1/sqrt(HD)

_CACHE = {}


def _build_nc():
    nc = bacc.Bacc()
    dt = mybir.dt

    xT = nc.dram_tensor("xT", [NFIN, NQ, 128, 3 * QTS], BF, kind="ExternalInput")
    cT = nc.dram_tensor("cT", [NDC, 3, 128, Lc], BF, kind="ExternalInput")
    w_d = {}
    for n, nch in (("wq", NFIN), ("wk", NDC), ("wv", NDC)):
        w_d[n] = nc.dram_tensor(n, [nch, 128, 3 * FS], BF, kind="ExternalInput")
    for n in ("wo1", "wo2"):
        w_d[n] = nc.dram_tensor(n, [HL, 128, F], BF, kind="ExternalInput")
    mb_d = nc.dram_tensor("mb", [128, NKT], F32, kind="ExternalInput")
    yr_d = nc.dram_tensor("yr", [S, F], BF, kind="ExternalOutput")
    yi_d = nc.dram_tensor("yi", [S, F], BF, kind="ExternalOutput")

    EXP = mybir.ActivationFunctionType.Exp

    with tile.TileContext(nc) as tc:
        with (
            tc.tile_pool(name="res", bufs=1) as res,       # kernel-lifetime tiles
            tc.tile_pool(name="xs", bufs=10) as xs,        # streamed xT slices
            tc.tile_pool(name="ep", bufs=14) as ep,        # exp(scores) tiles
            tc.tile_pool(name="dt", bufs=6) as dtp,        # dn add-tree tiles
            tc.tile_pool(name="rc", bufs=3) as rc,         # reciprocal staging
            tc.tile_pool(name="ys", bufs=4) as ys,         # y staging
            tc.tile_pool(name="ps", bufs=4, space="PSUM") as ps,
            tc.tile_pool(name="acc", bufs=2, space="PSUM") as acc,
            tc.tile_pool(name="yp", bufs=2, space="PSUM") as yp,
        ):
            def rtile(shape, dtype, tag):
                return res.tile(shape, dtype, tag=tag, name=tag)

            # K-proj inputs split across the sync HWDGE and gpsimd SWDGE
            # queues, component-major (m1 chains consume component 0 first);
            # wv/wq follow on gpsimd, wo last (needed only later).
            wk_sb = rtile([128, NDC, 3, FS], BF, "wk")
            cT_sb = rtile([128, NDC, 3, Lc], BF, "cT")

            rr = [nc.sync, nc.scalar, nc.gpsimd]
            rri = [0]

            def rr_dma(dst, src):
                rr[rri[0] % 3].dma_start(dst, src)
                rri[0] += 1

            # interleaved in K-proj consumption order, round-robin over the
            # three queues so arrivals track the m1 chain
            for c in range(NDC):
                rr_dma(wk_sb[:, c], w_d["wk"][c].rearrange("p (s w) -> p s w", s=3))
                rr_dma(cT_sb[:, c, 0], cT[c, 0])
            for s in (1, 2):
                for c in range(NDC):
                    rr_dma(cT_sb[:, c, s], cT[c, s])
            wv_sb = rtile([128, NDC, 3, FS], BF, "wv")
            for c in range(NDC):
                rr_dma(wv_sb[:, c], w_d["wv"][c].rearrange("p (s w) -> p s w", s=3))
            mb = rtile([128, NKT], F32, "mb")
            nc.gpsimd.dma_start(mb[:], mb_d[:])
            wq_sb = rtile([128, NFIN, 3, FS], BF, "wq")
            for c in range(NFIN):
                eng = nc.scalar if c % 2 == 0 else nc.gpsimd
                eng.dma_start(wq_sb[:, c], w_d["wq"][c].rearrange("p (s w) -> p s w", s=3))
            wo_sb = {}
            for n in ("wo1", "wo2"):
                t = rtile([128, HL * F], BF, n)
                for h in range(HL):
                    nc.gpsimd.dma_start(t[:, h * F : (h + 1) * F], w_d[n][h])
                wo_sb[n] = t

            ones128 = rtile([128, 128], BF, "ones128")
            nc.vector.memset(ones128[:], 1.0)

            QX = {h: rtile([128, S], BF, f"qx{h}") for h in range(HL)}
            KX = {h: rtile([128, Lc], BF, f"kx{h}") for h in range(HL)}
            Vsb = {kt: rtile([128, WW], BF, f"v{kt}") for kt in range(NKT)}
            OT = {h: rtile([128, S], BF, f"ot{h}") for h in range(HL)}

            def hsl(h):
                return slice(h * 128, (h + 1) * 128)

            # Gauss 3-mult complex projection: m1 = ar@Wr, m2 = ai@Wi,
            # m3 = (ar+ai)@(Wr+Wi); real = m1-m2, imag = m3-m1-m2. The (r+i)
            # components are precomputed host-side, so each complex GEMM costs
            # 3 real matmuls instead of 4 (the DVE absorbs the combines).
            def gauss_combine(dst, pair, qs, m):
                # dst rows: [real_h(64); imag_h(64)] per head of the pair.
                # DVE ops may read at most ONE psum operand, so m2 stages
                # through SBUF via the scalar engine first.
                w = qs.stop - qs.start
                m2c = rc.tile([128, w], F32, tag="rc", name="rc")
                nc.scalar.copy(m2c[:], m[1][:])
                u = rc.tile([128, w], F32, tag="rc", name="rc")
                nc.vector.tensor_sub(u[:], m[2][:], m2c[:])
                for j, h in enumerate((2 * pair, 2 * pair + 1)):
                    rows = slice(j * 64, j * 64 + 64)
                    nc.vector.tensor_sub(dst[h][0:64, qs], m[0][rows, :], m2c[rows, :])
                    nc.vector.tensor_sub(dst[h][64:128, qs], u[rows, :], m[0][rows, :])

            # ---- K projection ------------------------------------------------
            for kq in range(2):
                ks = slice(kq * 512, (kq + 1) * 512)
                for pair in range(2):
                    psl = slice(pair * 128, (pair + 1) * 128)
                    m = []
                    for s in range(3):
                        ac = ps.tile([128, QTS], F32, tag="ps", name="ps")
                        for c in range(NDC):
                            nc.tensor.matmul(
                                ac[:], wk_sb[:, c, s, psl], cT_sb[:, c, s, ks],
                                start=(c == 0), stop=(c == NDC - 1),
                            )
                        m.append(ac)
                    gauss_combine(KX, pair, ks, m)

            # ---- V projection (combines write the merged [vr|vi] columns) ---
            for kt in range(NKT):
                ksl = slice(kt * 128, (kt + 1) * 128)
                m = []
                for s in range(3):
                    ac = ps.tile([128, FS], F32, tag="ps", name="ps")
                    for c in range(NDC):
                        nc.tensor.matmul(
                            ac[:], cT_sb[:, c, s, ksl], wv_sb[:, c, s],
                            start=(c == 0), stop=(c == NDC - 1),
                        )
                    m.append(ac)
                m2c = rc.tile([128, FS], F32, tag="rc", name="rc")
                nc.scalar.copy(m2c[:], m[1][:])
                u = rc.tile([128, FS], F32, tag="rc", name="rc")
                nc.vector.tensor_sub(u[:], m[2][:], m2c[:])
                vview = Vsb[kt][:].rearrange("p (h d) -> p h d", h=HL)
                m0v = m[0][:].rearrange("p (h d) -> p h d", h=HL)
                m2v = m2c[:].rearrange("p (h d) -> p h d", h=HL)
                uv = u[:].rearrange("p (h d) -> p h d", h=HL)
                nc.vector.tensor_sub(vview[:, :, 0:64], m0v, m2v)
                nc.vector.tensor_sub(vview[:, :, 64:128], uv, m0v)

            # ---- Q projection (emits DMA + 2 pair-units for one q tile) -----
            def qproj_chains(q):
                qs = slice(q * QTS, (q + 1) * QTS)
                xt = {}
                for c in range(NFIN):
                    t = xs.tile([128, 3, QTS], BF, tag="xt", name="xt")
                    nc.sync.dma_start(t[:], xT[c, q].rearrange("p (s n) -> p s n", s=3))
                    xt[c] = t

                def pair_unit(pair):
                    psl = slice(pair * 128, (pair + 1) * 128)
                    m = []
                    for s in range(3):
                        ac = ps.tile([128, QTS], F32, tag="ps", name="ps")
                        for c in range(NFIN):
                            nc.tensor.matmul(
                                ac[:], wq_sb[:, c, s, psl], xt[c][:, s],
                                start=(c == 0), stop=(c == NFIN - 1),
                            )
                        m.append(ac)
                    gauss_combine(QX, pair, qs, m)

                return [lambda p=p: pair_unit(p) for p in range(2)]

            for q in (0, 1):
                for f in qproj_chains(q):
                    f()

            # ---- attention + out-proj, interleaved at chain granularity ------
            # PE program order alternates (2 score mms) with one independent
            # filler chain (out-proj of q-1, or Q-proj of q2/q3 during q0) so
            # the PE keeps streaming while the Scalar engine chews exp().
            def sc_pair(qoff, width, h, kp):
                qs = slice(qoff, qoff + width)
                es = []
                for j in range(2):
                    kt = 2 * kp + j
                    sp = ps.tile([128, width], F32, tag="ps", name="ps")
                    nc.tensor.matmul(
                        sp[:], KX[h][:, kt * 128 : (kt + 1) * 128],
                        QX[h][:, qs], start=True, stop=True,
                    )
                    e = ep.tile([128, width], BF, tag="e", name="e")
                    nc.scalar.activation(
                        e[:], sp[:], EXP, bias=mb[:, kt : kt + 1], scale=SCALE,
                    )
                    es.append(e)
                return es

            def dnav_h(qoff, width, h, e_list):
                qs = slice(qoff, qoff + width)
                # av on the PE; dn via DVE pairwise adds + one ones-matmul
                av = acc.tile([128, width], F32, tag="acc", name="acc")
                for kt in range(NKT):
                    nc.tensor.matmul(
                        av[:], Vsb[kt][:, hsl(h)], e_list[kt][:],
                        start=(kt == 0), stop=(kt == NKT - 1),
                    )
                lvl = e_list
                while len(lvl) > 1:
                    nxt = []
                    for i in range(0, len(lvl), 2):
                        s = dtp.tile([128, width], BF, tag="dt", name="dt")
                        nc.vector.tensor_add(s[:], lvl[i][:], lvl[i + 1][:])
                        nxt.append(s)
                    lvl = nxt
                dn = acc.tile([128, width], F32, tag="acc", name="acc")
                nc.tensor.matmul(dn[:], ones128[:], lvl[0][:], start=True, stop=True)
                rec = rc.tile([128, width], F32, tag="rc", name="rc")
                nc.vector.reciprocal_approx_fast(rec[:], dn[:])
                nc.vector.tensor_mul(OT[h][:, qs], av[:], rec[:])

            def op_unit(qi, wname, dram, eng):
                # both 512-col halves of one output row block -> one
                # contiguous 256KB DMA; queues rotated to spread the drain
                st = ys.tile([128, F], BF, tag="y", name="y")
                for fo in range(2):
                    ac = yp.tile([128, 512], F32, tag="yp", name="yp")
                    for h in range(HL):
                        nc.tensor.matmul(
                            ac[:], OT[h][:, qi * 128 : (qi + 1) * 128],
                            wo_sb[wname][:, h * F + fo * 512 : h * F + (fo + 1) * 512],
                            start=(h == 0), stop=(h == HL - 1),
                        )
                    nc.vector.tensor_copy(st[:, fo * 512 : (fo + 1) * 512], ac[:])
                eng.dma_start(dram[qi * 128 : (qi + 1) * 128, :], st[:])

            ENGS = [None]

            def op_chains(qoff, width):
                def unit(qi, wn, dr):
                    ENGS[0] = {nc.sync: nc.scalar, nc.scalar: nc.gpsimd}.get(
                        ENGS[0], nc.sync
                    )
                    op_unit(qi, wn, dr, ENGS[0])

                return [
                    lambda qi=qi, wn=wn, dr=dr: unit(qi, wn, dr)
                    for qi in range(qoff // 128, (qoff + width) // 128)
                    for wn, dr in (("wo1", yr_d), ("wo2", yi_d))
                ]

            # the last q tile is split in half so the final out-proj batch
            # (whose y DMAs can't start before the last head's softmax) is
            # 1MB instead of 2MB of end-bunched writes
            passes = [(0, 512), (512, 512), (1024, 512), (1536, 256), (1792, 256)]
            fillers = qproj_chains(2) + qproj_chains(3)
            for qoff, width in passes:
                e_q = {}
                nslot = HL * (NKT // 2)
                per_slot = [len(fillers) * (s + 1) // nslot for s in range(nslot)]
                done = 0
                for h in range(HL):
                    e_q[h] = []
                    for kp in range(NKT // 2):
                        e_q[h] += sc_pair(qoff, width, h, kp)
                        slot = h * (NKT // 2) + kp
                        while done < per_slot[slot]:
                            fillers[done]()
                            done += 1
                    if h > 0:
                        dnav_h(qoff, width, h - 1, e_q[h - 1])
                dnav_h(qoff, width, HL - 1, e_q[HL - 1])
                fillers = op_chains(qoff, width)
            for f in fillers:
                f()

    nc.compile()
    return nc


def _prep_in_maps(inputs):
    f32 = np.float32

    def bf(a):
        return np.ascontiguousarray(a).astype(BF16)

    x_r, x_i = np.asarray(inputs["x_r"], f32), np.asarray(inputs["x_i"], f32)
    ctx_r, ctx_i = np.asarray(inputs["ctx_r"], f32), np.asarray(inputs["ctx_i"], f32)
    mask = np.asarray(inputs["mask"], f32)
    W = {k: np.asarray(inputs[k], f32) for k in
         ("Wqr", "Wqi", "Wkr", "Wki", "Wvr", "Wvi", "Wor", "Woi")}

    per_batch = {}
    for b in range(B):
        def xtile(a):
            # [S, F] -> [F, S] -> [NFIN, NQ, 128, 512]
            return a.T.reshape(NFIN, 128, NQ, QTS).transpose(0, 2, 1, 3)

        # [NFIN, NQ, 128, 3, 512]: (r, i, r+i) per q tile (Gauss components)
        xri = np.stack(
            [xtile(x_r[b]), xtile(x_i[b]), xtile(x_r[b] + x_i[b])], axis=3
        )
        # [NDC, 3, 128, 1024]
        cri = np.stack(
            [
                ctx_r[b].T.reshape(NDC, 128, Lc),
                ctx_i[b].T.reshape(NDC, 128, Lc),
                (ctx_r[b] + ctx_i[b]).T.reshape(NDC, 128, Lc),
            ],
            axis=1,
        )
        per_batch[b] = {
            "xT": bf(xri.reshape(NFIN, NQ, 128, 3 * QTS)),
            "cT": bf(cri),
            "mb": np.ascontiguousarray(
                ((1.0 - mask[b]) * -1e9).astype(f32).reshape(NKT, 128).T
            ),
        }

    in_maps = []
    for core in range(NCORES):
        b, g = core // TPG, core % TPG
        m = dict(per_batch[b])
        gsl = slice(g * FS, (g + 1) * FS)
        for pre, wr, wi, nch in (
            ("wq", "Wqr", "Wqi", NFIN),
            ("wk", "Wkr", "Wki", NDC),
            ("wv", "Wvr", "Wvi", NDC),
        ):
            wrc, wic = W[wr][:, gsl], W[wi][:, gsl]
            # [nch, 128, 3, FS]: Gauss components (Wr, Wi, Wr+Wi)
            wri = np.stack(
                [
                    wrc.reshape(nch, 128, FS),
                    wic.reshape(nch, 128, FS),
                    (wrc + wic).reshape(nch, 128, FS),
                ],
                axis=2,
            )
            m[pre] = bf(wri.reshape(nch, 128, 3 * FS))
        # Wo: rows re-ordered to the merged [out_r_h(64); out_i_h(64)] layout.
        wo1 = np.empty((HL, 128, F), f32)
        wo2 = np.empty((HL, 128, F), f32)
        for h in range(HL):
            rs = slice(g * FS + h * HD, g * FS + (h + 1) * HD)
            wo1[h, :64] = W["Wor"][rs]
            wo1[h, 64:] = -W["Woi"][rs]
            wo2[h, :64] = W["Woi"][rs]
            wo2[h, 64:] = W["Wor"][rs]
        m["wo1"] = bf(wo1)
        m["wo2"] = bf(wo2)
        in_maps.append(m)
    return in_maps


def kernel(**inputs):
    if "nc" not in _CACHE:
        _CACHE["nc"] = _build_nc()
    nc = _CACHE["nc"]
    in_maps = _prep_in_maps(inputs)
    res = run_bass_kernel_spmd(nc, in_maps, core_ids=list(range(NCORES)))
    y = np.zeros((B, S, F), np.complex64)
    for core in range(NCORES):
        b = core // TPG
        y[b] += res.results[core]["yr"].astype(np.float32)
        y[b] += 1j * res.results[core]["yi"].astype(np.float32)
    return y
